# revision 1
# baseline (speedup 1.0000x reference)
"""Trainium2 Bass kernel for nn_Aggregationlayer (GNN message-passing aggregation).

Computes, for N=50000 nodes with K=16 mailbox slots and H=256 features:
    coord = clip(x) + mean_k(clip(trans))                  [N, 3]
    ef    = sum_k(edge_feature)                            [N, 256]
    h     = hh + (silu([hh, ef] @ W1 + b1) @ W2 + b2)      [N, 256]

Data-parallel over nodes on 8 NeuronCores (6250 nodes/core, padded to
6272 = 49*128). Per 128-node tile:
  - DMA the [128, 16, 256] mailbox slab (fp32, contiguous 16KB/partition)
  - DVE pairwise pre-sum k -> 8 groups, then PE transpose-accumulates the
    8 groups into PSUM, yielding ef^T [h, node] exactly in fp32
  - hh^T via PE transpose; MLP runs feature-major in fp32r (1 cyc/row at
    free dim 256) over 2-tile macros; silu on ACT with b1 as per-partition
    bias; b2 added via a K=1 ones^T@b2 matmul into the PSUM accumulation
  - residual add on DVE, coord path on DVE from resident trans/x tiles
"""

import numpy as np
from contextlib import ExitStack

import concourse.bacc as bacc
import concourse.tile as tile
import concourse.mybir as mybir
from concourse.bass_utils import run_bass_kernel_spmd

F32 = mybir.dt.float32
F32R = mybir.dt.float32r
AF = mybir.ActivationFunctionType
ALU = mybir.AluOpType

N = 50000
K = 16
H = 256
NCORES = 8
NPC = N // NCORES          # 6250 nodes per core
T = 49                     # tiles of 128 nodes per core
P = 128
NPAD = T * P               # 6272 padded nodes per core
CLIP = 1000.0


def build_module():
    nc = bacc.Bacc(None, target_bir_lowering=False)

    x_d = nc.dram_tensor("x", [NPAD, 3], F32, kind="ExternalInput")
    tr_d = nc.dram_tensor("trans", [NPAD, K, 3], F32, kind="ExternalInput")
    e_d = nc.dram_tensor("edge", [NPAD, K, H], F32, kind="ExternalInput")
    hh_d = nc.dram_tensor("hh", [NPAD, H], F32, kind="ExternalInput")
    w1_d = nc.dram_tensor("w1", [2 * H, H], F32, kind="ExternalInput")
    b1_d = nc.dram_tensor("b1", [H], F32, kind="ExternalInput")
    w2_d = nc.dram_tensor("w2", [H, H], F32, kind="ExternalInput")
    b2_d = nc.dram_tensor("b2", [H], F32, kind="ExternalInput")
    id_d = nc.dram_tensor("ident", [P, P], F32, kind="ExternalInput")
    on_d = nc.dram_tensor("ones1", [1, P], F32, kind="ExternalInput")

    co_d = nc.dram_tensor("coord", [NPAD, 3], F32, kind="ExternalOutput")
    h_d = nc.dram_tensor("hout", [NPAD, H], F32, kind="ExternalOutput")

    with tile.TileContext(nc) as tc, ExitStack() as ctx:
        singles = ctx.enter_context(tc.tile_pool(name="singles", bufs=1))
        slab_p = ctx.enter_context(tc.tile_pool(name="slab", bufs=4))
        hh_p = ctx.enter_context(tc.tile_pool(name="hhp", bufs=4))
        tmp_p = ctx.enter_context(tc.tile_pool(name="tmp", bufs=3))
        agg_p = ctx.enter_context(tc.tile_pool(name="agg", bufs=2))
        act_p = ctx.enter_context(tc.tile_pool(name="act", bufs=2))
        out_p = ctx.enter_context(tc.tile_pool(name="outp", bufs=4))
        crd_p = ctx.enter_context(tc.tile_pool(name="crd", bufs=2))
        ps_tr = ctx.enter_context(tc.tile_pool(name="ps_tr", bufs=3, space="PSUM"))
        ps_p1 = ctx.enter_context(tc.tile_pool(name="ps_p1", bufs=2, space="PSUM"))
        ps_p2 = ctx.enter_context(tc.tile_pool(name="ps_p2", bufs=3, space="PSUM"))

        # ---- one-time preloads ----
        id_sb = singles.tile([P, P], F32)
        nc.sync.dma_start(out=id_sb, in_=id_d[:])
        w1r = singles.tile([P, 4, H], F32R)
        nc.gpsimd.dma_start(out=w1r, in_=w1_d.rearrange("(c p) m -> p c m", p=P))
        w2r = singles.tile([P, 2, H], F32R)
        nc.gpsimd.dma_start(out=w2r, in_=w2_d.rearrange("(c p) m -> p c m", p=P))
        b1_sb = singles.tile([P, 2], F32)
        nc.sync.dma_start(out=b1_sb, in_=b1_d.rearrange("(c p) -> p c", p=P))
        b2r = singles.tile([1, H], F32R)
        nc.gpsimd.dma_start(out=b2r, in_=b2_d[:].unsqueeze(0))
        ones_r = singles.tile([1, P], F32R)
        nc.gpsimd.dma_start(out=ones_r, in_=on_d[:])
        trans_all = singles.tile([P, T, K * 3], F32)
        nc.sync.dma_start(
            out=trans_all, in_=tr_d.rearrange("(t p) k c -> p t (k c)", p=P)
        )
        x_all = singles.tile([P, T, 3], F32)
        nc.sync.dma_start(out=x_all, in_=x_d.rearrange("(t p) c -> p t c", p=P))
        coord_all = singles.tile([P, T, 3], F32)

        hh_tiles = {}

        def load_and_reduce(t, aggT_r, tl):
            """DMA tile t, K-reduce, transpose; fills aggT_r[:, :, tl*P:(tl+1)*P]."""
            slab = slab_p.tile([P, K, H], F32, tag="slab")
            nc.sync.dma_start(out=slab, in_=e_d[t * P:(t + 1) * P])
            hh_sb = hh_p.tile([P, H], F32, tag="hh")
            nc.sync.dma_start(out=hh_sb, in_=hh_d[t * P:(t + 1) * P, :])
            hh_tiles[t] = hh_sb

            tmp8 = tmp_p.tile([P, 8, H], F32, tag="tmp8")
            nc.vector.tensor_tensor(
                out=tmp8[:], in0=slab[:, 0:K:2, :], in1=slab[:, 1:K:2, :], op=ALU.add
            )
            ns = slice(tl * P, (tl + 1) * P)
            for c in range(2):
                pt = ps_tr.tile([P, P], F32, tag="pt")
                for g in range(8):
                    nc.tensor.matmul(
                        pt[:], tmp8[:, g, c * P:(c + 1) * P], id_sb[:],
                        is_transpose=True, start=(g == 0), stop=(g == 7),
                    )
                nc.scalar.copy(aggT_r[:, 2 + c, ns], pt[:])
            for c in range(2):
                pt = ps_tr.tile([P, P], F32, tag="pt")
                nc.tensor.transpose(pt[:], hh_sb[:, c * P:(c + 1) * P], id_sb[:])
                nc.scalar.copy(aggT_r[:, c, ns], pt[:])

        def mlp_and_store(tiles, aggT_r):
            """MLP over a macro of 1-2 tiles; residual; store h rows."""
            W = len(tiles) * P
            actT_r = act_p.tile([P, 2, 2 * P], F32R, tag="actT")
            for half in range(2):
                p1 = ps_p1.tile([P, 2 * P], F32, tag="p1")
                for c in range(4):
                    nc.tensor.matmul(
                        p1[:, :W],
                        w1r[:, c, half * P:(half + 1) * P],
                        aggT_r[:, c, :W],
                        start=(c == 0), stop=(c == 3),
                    )
                nc.scalar.activation(
                    actT_r[:, half, :W], p1[:, :W], AF.Silu,
                    bias=b1_sb[:, half:half + 1], scale=1.0,
                )
            for tl, t in enumerate(tiles):
                p2 = ps_p2.tile([P, H], F32, tag="p2")
                ns = slice(tl * P, (tl + 1) * P)
                for c in range(2):
                    nc.tensor.matmul(
                        p2[:], actT_r[:, c, ns], w2r[:, c, :],
                        start=(c == 0), stop=False,
                    )
                nc.tensor.matmul(p2[:], ones_r[:], b2r[:], start=False, stop=True)
                out_sb = out_p.tile([P, H], F32, tag="out")
                nc.vector.tensor_tensor(
                    out=out_sb[:], in0=p2[:], in1=hh_tiles.pop(t)[:], op=ALU.add
                )
                nc.scalar.dma_start(out=h_d[t * P:(t + 1) * P, :], in_=out_sb[:])

        def coord_group(g0, g1):
            """coord for tiles [g0, g1): clip, mean over k, add clip(x)."""
            n = g1 - g0
            tr4 = trans_all[:, g0:g1, :].rearrange("p t (k c) -> p t k c", c=3)
            tc4 = crd_p.tile([P, n, K, 3], F32, tag="tc")
            nc.vector.tensor_scalar(
                out=tc4[:], in0=tr4, scalar1=-CLIP, scalar2=CLIP,
                op0=ALU.max, op1=ALU.min,
            )
            t8 = crd_p.tile([P, n, 8, 3], F32, tag="t8")
            nc.vector.tensor_tensor(
                out=t8[:], in0=tc4[:, :, 0:16:2, :], in1=tc4[:, :, 1:16:2, :], op=ALU.add
            )
            t4 = crd_p.tile([P, n, 4, 3], F32, tag="t4")
            nc.vector.tensor_tensor(
                out=t4[:], in0=t8[:, :, 0:8:2, :], in1=t8[:, :, 1:8:2, :], op=ALU.add
            )
            t2 = crd_p.tile([P, n, 2, 3], F32, tag="t2")
            nc.vector.tensor_tensor(
                out=t2[:], in0=t4[:, :, 0:4:2, :], in1=t4[:, :, 1:4:2, :], op=ALU.add
            )
            t1 = crd_p.tile([P, n, 3], F32, tag="t1")
            nc.vector.tensor_tensor(
                out=t1[:], in0=t2[:, :, 0, :], in1=t2[:, :, 1, :], op=ALU.add
            )
            nc.vector.tensor_scalar(
                out=t1[:], in0=t1[:], scalar1=1.0 / K, scalar2=None, op0=ALU.mult
            )
            xc = crd_p.tile([P, n, 3], F32, tag="xc")
            nc.vector.tensor_scalar(
                out=xc[:], in0=x_all[:, g0:g1, :], scalar1=-CLIP, scalar2=CLIP,
                op0=ALU.max, op1=ALU.min,
            )
            nc.vector.tensor_tensor(
                out=coord_all[:, g0:g1, :], in0=t1[:], in1=xc[:], op=ALU.add
            )

        # ---- main loop: macros of 2 tiles (last macro is 1 tile) ----
        for m in range((T + 1) // 2):
            tiles = [t for t in (2 * m, 2 * m + 1) if t < T]
            aggT_r = agg_p.tile([P, 4, 2 * P], F32R, tag="aggT")
            for tl, t in enumerate(tiles):
                load_and_reduce(t, aggT_r, tl)
            mlp_and_store(tiles, aggT_r)
            if m % 2 == 1:
                coord_group(4 * (m // 2), min(4 * (m // 2) + 4, T))
        if T % 4 != 0:
            coord_group(T - T % 4, T)

        nc.scalar.dma_start(
            out=co_d.rearrange("(t p) c -> p t c", p=P), in_=coord_all[:]
        )

    nc.compile()
    return nc


_NC_CACHE = []


def get_module():
    if not _NC_CACHE:
        _NC_CACHE.append(build_module())
    return _NC_CACHE[0]


def make_in_maps(x, trans, edge_feature, hh, W1, b1, W2, b2):
    ident = np.eye(P, dtype=np.float32)
    ones1 = np.ones((1, P), np.float32)
    in_maps = []
    for c in range(NCORES):
        sl = slice(c * NPC, (c + 1) * NPC)

        def pad(a):
            out = np.zeros((NPAD,) + a.shape[1:], np.float32)
            out[:NPC] = a[sl]
            return out

        in_maps.append({
            "x": pad(x), "trans": pad(trans), "edge": pad(edge_feature),
            "hh": pad(hh), "w1": np.ascontiguousarray(W1, np.float32),
            "b1": np.ascontiguousarray(b1, np.float32),
            "w2": np.ascontiguousarray(W2, np.float32),
            "b2": np.ascontiguousarray(b2, np.float32),
            "ident": ident, "ones1": ones1,
        })
    return in_maps


def kernel(x, trans, edge_feature, hh, W1, b1, W2, b2):
    nc = get_module()
    in_maps = make_in_maps(x, trans, edge_feature, hh, W1, b1, W2, b2)
    res = run_bass_kernel_spmd(nc, in_maps, core_ids=list(range(NCORES)))
    coord = np.empty((N, 3), np.float32)
    h = np.empty((N, H), np.float32)
    for c in range(NCORES):
        sl = slice(c * NPC, (c + 1) * NPC)
        coord[sl] = res.results[c]["coord"][:NPC]
        h[sl] = res.results[c]["hout"][:NPC]
    return coord, h


# revision 8
# speedup vs baseline: 1.0004x; 1.0004x over previous
"""Trainium2 Bass kernel for nn_Aggregationlayer (GNN message-passing aggregation).

Computes, for N=50000 nodes with K=16 mailbox slots and H=256 features:
    coord = clip(x) + mean_k(clip(trans))                  [N, 3]
    ef    = sum_k(edge_feature)                            [N, 256]
    h     = hh + (silu([hh, ef] @ W1 + b1) @ W2 + b2)      [N, 256]

Data-parallel over nodes on 8 NeuronCores (6250 nodes/core, padded to
6272 = 49*128). Per 128-node tile:
  - DMA the [128, 16, 256] mailbox slab (fp32, contiguous 16KB/partition)
  - DVE pairwise pre-sum k -> 8 groups, then PE transpose-accumulates the
    8 groups into PSUM, yielding ef^T [h, node] exactly in fp32
  - hh^T via PE transpose; MLP runs feature-major in fp32r (1 cyc/row at
    free dim 256) over 2-tile macros; silu on ACT with b1 as per-partition
    bias; b2 added via a K=1 ones^T@b2 matmul into the PSUM accumulation
  - residual add on DVE, coord path on DVE from resident trans/x tiles
"""

import numpy as np
from contextlib import ExitStack

import concourse.bacc as bacc
import concourse.tile as tile
import concourse.mybir as mybir
from concourse.bass_utils import run_bass_kernel_spmd

F32 = mybir.dt.float32
F32R = mybir.dt.float32r
AF = mybir.ActivationFunctionType
ALU = mybir.AluOpType

N = 50000
K = 16
H = 256
NCORES = 8
NPC = N // NCORES          # 6250 nodes per core
T = 49                     # tiles of 128 nodes per core
P = 128
NPAD = T * P               # 6272 padded nodes per core
CLIP = 1000.0


def build_module(variant="full"):
    """variant: "full" (the real kernel), "dma" (DMA traffic only, bogus
    compute skipped), "compute" (edge DMA replaced by one resident slab).
    Non-full variants produce wrong outputs; they exist for perf probes."""
    nc = bacc.Bacc(None, target_bir_lowering=False)

    x_d = nc.dram_tensor("x", [NPAD, 3], F32, kind="ExternalInput")
    tr_d = nc.dram_tensor("trans", [NPAD, K, 3], F32, kind="ExternalInput")
    e_d = nc.dram_tensor("edge", [NPAD, K, H], F32, kind="ExternalInput")
    hh_d = nc.dram_tensor("hh", [NPAD, H], F32, kind="ExternalInput")
    w1_d = nc.dram_tensor("w1", [2 * H, H], F32, kind="ExternalInput")
    b1_d = nc.dram_tensor("b1", [H], F32, kind="ExternalInput")
    w2_d = nc.dram_tensor("w2", [H, H], F32, kind="ExternalInput")
    b2_d = nc.dram_tensor("b2", [H], F32, kind="ExternalInput")
    id_d = nc.dram_tensor("ident", [P, P], F32, kind="ExternalInput")
    on_d = nc.dram_tensor("ones1", [1, P], F32, kind="ExternalInput")

    co_d = nc.dram_tensor("coord", [NPAD, 3], F32, kind="ExternalOutput")
    h_d = nc.dram_tensor("hout", [NPAD, H], F32, kind="ExternalOutput")

    with tile.TileContext(nc) as tc, ExitStack() as ctx:
        singles = ctx.enter_context(tc.tile_pool(name="singles", bufs=1))
        slab_p = ctx.enter_context(tc.tile_pool(name="slab", bufs=4))
        hh_p = ctx.enter_context(tc.tile_pool(name="hhp", bufs=4))
        tmp_p = ctx.enter_context(tc.tile_pool(name="tmp", bufs=3))
        agg_p = ctx.enter_context(tc.tile_pool(name="agg", bufs=2))
        act_p = ctx.enter_context(tc.tile_pool(name="act", bufs=2))
        out_p = ctx.enter_context(tc.tile_pool(name="outp", bufs=4))
        crd_p = ctx.enter_context(tc.tile_pool(name="crd", bufs=2))
        ps_tr = ctx.enter_context(tc.tile_pool(name="ps_tr", bufs=3, space="PSUM"))
        ps_p1 = ctx.enter_context(tc.tile_pool(name="ps_p1", bufs=2, space="PSUM"))
        ps_p2 = ctx.enter_context(tc.tile_pool(name="ps_p2", bufs=3, space="PSUM"))

        # ---- one-time preloads ----
        id_sb = singles.tile([P, P], F32)
        nc.sync.dma_start(out=id_sb, in_=id_d[:])
        idr_sb = singles.tile([P, P], F32R)
        nc.gpsimd.dma_start(out=idr_sb, in_=id_d[:])
        w1r = singles.tile([P, 4, H], F32R)
        nc.gpsimd.dma_start(out=w1r, in_=w1_d.rearrange("(c p) m -> p c m", p=P))
        w2r = singles.tile([P, 2, H], F32R)
        nc.gpsimd.dma_start(out=w2r, in_=w2_d.rearrange("(c p) m -> p c m", p=P))
        b1_sb = singles.tile([P, 2], F32)
        nc.sync.dma_start(out=b1_sb, in_=b1_d.rearrange("(c p) -> p c", p=P))
        b2r = singles.tile([1, H], F32R)
        nc.gpsimd.dma_start(out=b2r, in_=b2_d[:].unsqueeze(0))
        ones_r = singles.tile([1, P], F32R)
        nc.gpsimd.dma_start(out=ones_r, in_=on_d[:])
        trans_all = singles.tile([P, T, K * 3], F32)
        nc.sync.dma_start(
            out=trans_all, in_=tr_d.rearrange("(t p) k c -> p t (k c)", p=P)
        )
        x_all = singles.tile([P, T, 3], F32)
        nc.sync.dma_start(out=x_all, in_=x_d.rearrange("(t p) c -> p t c", p=P))
        coord_all = singles.tile([P, T, 3], F32)

        hh_tiles = {}

        shared_slab = None
        if variant == "compute":
            shared_slab = singles.tile([P, K, H], F32)
            nc.sync.dma_start(out=shared_slab, in_=e_d[0:P])

        def load_and_reduce(t, aggT_r, tl):
            """DMA tile t, K-reduce, transpose; fills aggT_r[:, :, tl*P:(tl+1)*P]."""
            if variant == "compute":
                slab = shared_slab
            else:
                slab = slab_p.tile([P, K, H], F32, tag="slab")
                nc.sync.dma_start(out=slab, in_=e_d[t * P:(t + 1) * P])
            hh_sb = hh_p.tile([P, H], F32, tag="hh")
            nc.sync.dma_start(out=hh_sb, in_=hh_d[t * P:(t + 1) * P, :])
            hh_tiles[t] = hh_sb
            if variant == "dma":
                out_sb = out_p.tile([P, H], F32, tag="out")
                nc.vector.tensor_copy(out_sb[:], hh_sb[:])
                nc.scalar.dma_start(out=h_d[t * P:(t + 1) * P, :], in_=out_sb[:])
                hh_tiles.pop(t)
                return

            tmp8 = tmp_p.tile([P, 8, H], F32, tag="tmp8")
            nc.vector.tensor_tensor(
                out=tmp8[:], in0=slab[:, 0:K:2, :], in1=slab[:, 1:K:2, :], op=ALU.add
            )
            tmp4 = tmp_p.tile([P, 4, H], F32R, tag="tmp4")
            nc.vector.tensor_tensor(
                out=tmp4[:], in0=tmp8[:, 0:8:2, :], in1=tmp8[:, 1:8:2, :], op=ALU.add
            )
            ns = slice(tl * P, (tl + 1) * P)
            # one PSUM bank holds all 4 aggT chunks: [hh0, hh1, ef0, ef1]
            pt4 = ps_tr.tile([P, 4, P], F32, tag="pt4")
            for c in range(2):
                nc.tensor.transpose(pt4[:, c, :], hh_sb[:, c * P:(c + 1) * P], id_sb[:])
            for c in range(2):
                for g in range(4):
                    nc.tensor.matmul(
                        pt4[:, 2 + c, :].bitcast(F32R),
                        tmp4[:, g, c * P:(c + 1) * P], idr_sb[:],
                        is_transpose=True, start=(g == 0), stop=(g == 3),
                    )
            nc.vector.tensor_copy(aggT_r[:, :, ns], pt4[:])

        def mlp_and_store(tiles, aggT_r):
            """MLP over a macro of 1-2 tiles; residual; store h rows."""
            W = len(tiles) * P
            actT_r = act_p.tile([P, 2, 2 * P], F32R, tag="actT")
            for half in range(2):
                p1 = ps_p1.tile([P, 2 * P], F32, tag="p1")
                for c in range(4):
                    nc.tensor.matmul(
                        p1[:, :W],
                        w1r[:, c, half * P:(half + 1) * P],
                        aggT_r[:, c, :W],
                        start=(c == 0), stop=(c == 3),
                    )
                nc.scalar.activation(
                    actT_r[:, half, :W], p1[:, :W], AF.Silu,
                    bias=b1_sb[:, half:half + 1], scale=1.0,
                )
            for tl, t in enumerate(tiles):
                p2 = ps_p2.tile([P, H], F32, tag="p2")
                ns = slice(tl * P, (tl + 1) * P)
                for c in range(2):
                    nc.tensor.matmul(
                        p2[:], actT_r[:, c, ns], w2r[:, c, :],
                        start=(c == 0), stop=False,
                    )
                nc.tensor.matmul(p2[:], ones_r[:], b2r[:], start=False, stop=True)
                out_sb = out_p.tile([P, H], F32, tag="out")
                nc.vector.tensor_tensor(
                    out=out_sb[:], in0=p2[:], in1=hh_tiles.pop(t)[:], op=ALU.add
                )
                nc.scalar.dma_start(out=h_d[t * P:(t + 1) * P, :], in_=out_sb[:])

        def coord_group(g0, g1):
            """coord for tiles [g0, g1): clip, mean over k, add clip(x)."""
            n = g1 - g0
            tr4 = trans_all[:, g0:g1, :].rearrange("p t (k c) -> p t k c", c=3)
            tc4 = crd_p.tile([P, n, K, 3], F32, tag="tc")
            nc.vector.tensor_scalar(
                out=tc4[:], in0=tr4, scalar1=-CLIP, scalar2=CLIP,
                op0=ALU.max, op1=ALU.min,
            )
            t8 = crd_p.tile([P, n, 8, 3], F32, tag="t8")
            nc.vector.tensor_tensor(
                out=t8[:], in0=tc4[:, :, 0:16:2, :], in1=tc4[:, :, 1:16:2, :], op=ALU.add
            )
            t4 = crd_p.tile([P, n, 4, 3], F32, tag="t4")
            nc.vector.tensor_tensor(
                out=t4[:], in0=t8[:, :, 0:8:2, :], in1=t8[:, :, 1:8:2, :], op=ALU.add
            )
            t2 = crd_p.tile([P, n, 2, 3], F32, tag="t2")
            nc.vector.tensor_tensor(
                out=t2[:], in0=t4[:, :, 0:4:2, :], in1=t4[:, :, 1:4:2, :], op=ALU.add
            )
            t1 = crd_p.tile([P, n, 3], F32, tag="t1")
            nc.vector.tensor_tensor(
                out=t1[:], in0=t2[:, :, 0, :], in1=t2[:, :, 1, :], op=ALU.add
            )
            nc.vector.tensor_scalar(
                out=t1[:], in0=t1[:], scalar1=1.0 / K, scalar2=None, op0=ALU.mult
            )
            xc = crd_p.tile([P, n, 3], F32, tag="xc")
            nc.vector.tensor_scalar(
                out=xc[:], in0=x_all[:, g0:g1, :], scalar1=-CLIP, scalar2=CLIP,
                op0=ALU.max, op1=ALU.min,
            )
            nc.vector.tensor_tensor(
                out=coord_all[:, g0:g1, :], in0=t1[:], in1=xc[:], op=ALU.add
            )

        # ---- main loop: macros of 2 tiles (last macro is 1 tile) ----
        for m in range((T + 1) // 2):
            tiles = [t for t in (2 * m, 2 * m + 1) if t < T]
            aggT_r = agg_p.tile([P, 4, 2 * P], F32R, tag="aggT")
            for tl, t in enumerate(tiles):
                load_and_reduce(t, aggT_r, tl)
            if variant != "dma":
                mlp_and_store(tiles, aggT_r)
                if m % 2 == 1:
                    coord_group(4 * (m // 2), min(4 * (m // 2) + 4, T))
        if variant != "dma" and T % 4 != 0:
            coord_group(T - T % 4, T)
        if variant == "dma":
            nc.vector.memset(coord_all[:], 0.0)

        nc.scalar.dma_start(
            out=co_d.rearrange("(t p) c -> p t c", p=P), in_=coord_all[:]
        )

    nc.compile()
    return nc


_NC_CACHE = []


def get_module():
    if not _NC_CACHE:
        _NC_CACHE.append(build_module())
    return _NC_CACHE[0]


def make_in_maps(x, trans, edge_feature, hh, W1, b1, W2, b2):
    ident = np.eye(P, dtype=np.float32)
    ones1 = np.ones((1, P), np.float32)
    in_maps = []
    for c in range(NCORES):
        sl = slice(c * NPC, (c + 1) * NPC)

        def pad(a):
            out = np.zeros((NPAD,) + a.shape[1:], np.float32)
            out[:NPC] = a[sl]
            return out

        in_maps.append({
            "x": pad(x), "trans": pad(trans), "edge": pad(edge_feature),
            "hh": pad(hh), "w1": np.ascontiguousarray(W1, np.float32),
            "b1": np.ascontiguousarray(b1, np.float32),
            "w2": np.ascontiguousarray(W2, np.float32),
            "b2": np.ascontiguousarray(b2, np.float32),
            "ident": ident, "ones1": ones1,
        })
    return in_maps


def kernel(x, trans, edge_feature, hh, W1, b1, W2, b2):
    nc = get_module()
    in_maps = make_in_maps(x, trans, edge_feature, hh, W1, b1, W2, b2)
    res = run_bass_kernel_spmd(nc, in_maps, core_ids=list(range(NCORES)))
    coord = np.empty((N, 3), np.float32)
    h = np.empty((N, H), np.float32)
    for c in range(NCORES):
        sl = slice(c * NPC, (c + 1) * NPC)
        coord[sl] = res.results[c]["coord"][:NPC]
        h[sl] = res.results[c]["hout"][:NPC]
    return coord, h


# revision 11
# speedup vs baseline: 2.0498x; 2.0490x over previous
"""Trainium2 Bass kernel for nn_Aggregationlayer (GNN message-passing aggregation).

Computes, for N=50000 nodes with K=16 mailbox slots and H=256 features:
    coord = clip(x) + mean_k(clip(trans))                  [N, 3]
    ef    = sum_k(edge_feature)                            [N, 256]
    h     = hh + (silu([hh, ef] @ W1 + b1) @ W2 + b2)      [N, 256]

Data-parallel over nodes on 8 NeuronCores (6250 nodes/core, padded to
6272 = 49*128). Per 128-node tile:
  - DMA the [128, 16, 256] mailbox slab (fp32, contiguous 16KB/partition)
  - DVE pairwise pre-sum k -> 8 groups, then PE transpose-accumulates the
    8 groups into PSUM, yielding ef^T [h, node] exactly in fp32
  - hh^T via PE transpose; MLP runs feature-major in fp32r (1 cyc/row at
    free dim 256) over 2-tile macros; silu on ACT with b1 as per-partition
    bias; b2 added via a K=1 ones^T@b2 matmul into the PSUM accumulation
  - residual add on DVE, coord path on DVE from resident trans/x tiles
"""

import numpy as np
from contextlib import ExitStack

import concourse.bacc as bacc
import concourse.tile as tile
import concourse.mybir as mybir
from concourse.bass_utils import run_bass_kernel_spmd

F32 = mybir.dt.float32
F32R = mybir.dt.float32r
AF = mybir.ActivationFunctionType
ALU = mybir.AluOpType

N = 50000
K = 16
H = 256
NCORES = 8
NPC = N // NCORES          # 6250 nodes per core
T = 49                     # tiles of 128 nodes per core
P = 128
NPAD = T * P               # 6272 padded nodes per core
CLIP = 1000.0


def build_module(variant="full", repeats=1):
    """variant: "full" (the real kernel), "dma" (DMA traffic only, bogus
    compute skipped), "compute" (edge DMA replaced by one resident slab).
    repeats>1 re-runs the main loop (perf probes: slope between repeat
    counts cancels dispatch overhead). Non-full/repeated variants exist
    only for probing."""
    nc = bacc.Bacc(None, target_bir_lowering=False)

    x_d = nc.dram_tensor("x", [NPAD, 3], F32, kind="ExternalInput")
    tr_d = nc.dram_tensor("trans", [NPAD, K, 3], F32, kind="ExternalInput")
    e_d = nc.dram_tensor("edge", [NPAD, K, H], F32, kind="ExternalInput")
    hh_d = nc.dram_tensor("hh", [NPAD, H], F32, kind="ExternalInput")
    w1_d = nc.dram_tensor("w1", [2 * H, H], F32, kind="ExternalInput")
    b1_d = nc.dram_tensor("b1", [H], F32, kind="ExternalInput")
    w2_d = nc.dram_tensor("w2", [H, H], F32, kind="ExternalInput")
    b2_d = nc.dram_tensor("b2", [H], F32, kind="ExternalInput")
    id_d = nc.dram_tensor("ident", [P, P], F32, kind="ExternalInput")
    on_d = nc.dram_tensor("ones1", [1, P], F32, kind="ExternalInput")

    co_d = nc.dram_tensor("coord", [NPAD, 3], F32, kind="ExternalOutput")
    h_d = nc.dram_tensor("hout", [NPAD, H], F32, kind="ExternalOutput")

    with tile.TileContext(nc) as tc, ExitStack() as ctx:
        singles = ctx.enter_context(tc.tile_pool(name="singles", bufs=1))
        slab_p = ctx.enter_context(tc.tile_pool(name="slab", bufs=6))
        hh_p = ctx.enter_context(tc.tile_pool(name="hhp", bufs=4))
        tmp_p = ctx.enter_context(tc.tile_pool(name="tmp", bufs=4))
        agg_p = ctx.enter_context(tc.tile_pool(name="agg", bufs=2))
        act_p = ctx.enter_context(tc.tile_pool(name="act", bufs=2))
        out_p = ctx.enter_context(tc.tile_pool(name="outp", bufs=4))
        crd_p = ctx.enter_context(tc.tile_pool(name="crd", bufs=2))
        ps_tr = ctx.enter_context(tc.tile_pool(name="ps_tr", bufs=3, space="PSUM"))
        ps_p1 = ctx.enter_context(tc.tile_pool(name="ps_p1", bufs=2, space="PSUM"))
        ps_p2 = ctx.enter_context(tc.tile_pool(name="ps_p2", bufs=3, space="PSUM"))

        # ---- one-time preloads ----
        id_sb = singles.tile([P, P], F32)
        nc.sync.dma_start(out=id_sb, in_=id_d[:])
        idr_sb = singles.tile([P, P], F32R)
        nc.gpsimd.dma_start(out=idr_sb, in_=id_d[:])
        w1r = singles.tile([P, 4, H], F32R)
        nc.gpsimd.dma_start(out=w1r, in_=w1_d.rearrange("(c p) m -> p c m", p=P))
        w2r = singles.tile([P, 2, H], F32R)
        nc.gpsimd.dma_start(out=w2r, in_=w2_d.rearrange("(c p) m -> p c m", p=P))
        b1_sb = singles.tile([P, 2], F32)
        nc.sync.dma_start(out=b1_sb, in_=b1_d.rearrange("(c p) -> p c", p=P))
        b2r = singles.tile([1, H], F32R)
        nc.gpsimd.dma_start(out=b2r, in_=b2_d[:].unsqueeze(0))
        ones_r = singles.tile([1, P], F32R)
        nc.gpsimd.dma_start(out=ones_r, in_=on_d[:])
        trans_all = singles.tile([P, T, K * 3], F32)
        nc.sync.dma_start(
            out=trans_all, in_=tr_d.rearrange("(t p) k c -> p t (k c)", p=P)
        )
        x_all = singles.tile([P, T, 3], F32)
        nc.sync.dma_start(out=x_all, in_=x_d.rearrange("(t p) c -> p t c", p=P))
        coord_all = singles.tile([P, T, 3], F32)

        hh_tiles = {}

        shared_slab = None
        if variant == "compute":
            shared_slab = singles.tile([P, K, H], F32)
            nc.sync.dma_start(out=shared_slab, in_=e_d[0:P])

        def load_and_reduce(t, aggT_r, tl):
            """DMA tile t, K-reduce, transpose; fills aggT_r[:, :, tl*P:(tl+1)*P]."""
            if variant == "compute":
                slab = shared_slab
            else:
                slab = slab_p.tile([P, K, H], F32, tag="slab")
                nc.sync.dma_start(out=slab, in_=e_d[t * P:(t + 1) * P])
            hh_sb = hh_p.tile([P, H], F32, tag="hh")
            nc.sync.dma_start(out=hh_sb, in_=hh_d[t * P:(t + 1) * P, :])
            hh_tiles[t] = hh_sb
            if variant == "dma":
                out_sb = out_p.tile([P, H], F32, tag="out")
                nc.vector.tensor_copy(out_sb[:], hh_sb[:])
                nc.scalar.dma_start(out=h_d[t * P:(t + 1) * P, :], in_=out_sb[:])
                hh_tiles.pop(t)
                return

            tmp8 = tmp_p.tile([P, 8, H], F32, tag="tmp8")
            nc.vector.tensor_tensor(
                out=tmp8[:], in0=slab[:, 0:K:2, :], in1=slab[:, 1:K:2, :], op=ALU.add
            )
            tmp4 = tmp_p.tile([P, 4, H], F32R, tag="tmp4")
            nc.vector.tensor_tensor(
                out=tmp4[:], in0=tmp8[:, 0:8:2, :], in1=tmp8[:, 1:8:2, :], op=ALU.add
            )
            ns = slice(tl * P, (tl + 1) * P)
            # one PSUM bank holds all 4 aggT chunks: [hh0, hh1, ef0, ef1]
            pt4 = ps_tr.tile([P, 4, P], F32, tag="pt4")
            for c in range(2):
                nc.tensor.transpose(pt4[:, c, :], hh_sb[:, c * P:(c + 1) * P], id_sb[:])
            for c in range(2):
                for g in range(4):
                    nc.tensor.matmul(
                        pt4[:, 2 + c, :].bitcast(F32R),
                        tmp4[:, g, c * P:(c + 1) * P], idr_sb[:],
                        is_transpose=True, start=(g == 0), stop=(g == 3),
                    )
            nc.scalar.copy(aggT_r[:, :, ns], pt4[:])

        def mlp_and_store(tiles, aggT_r):
            """MLP over a macro of 1-2 tiles; residual; store h rows."""
            W = len(tiles) * P
            actT_r = act_p.tile([P, 2, 2 * P], F32R, tag="actT")
            for half in range(2):
                p1 = ps_p1.tile([P, 2 * P], F32, tag="p1")
                for c in range(4):
                    nc.tensor.matmul(
                        p1[:, :W],
                        w1r[:, c, half * P:(half + 1) * P],
                        aggT_r[:, c, :W],
                        start=(c == 0), stop=(c == 3),
                    )
                nc.scalar.activation(
                    actT_r[:, half, :W], p1[:, :W], AF.Silu,
                    bias=b1_sb[:, half:half + 1], scale=1.0,
                )
            for tl, t in enumerate(tiles):
                p2 = ps_p2.tile([P, H], F32, tag="p2")
                ns = slice(tl * P, (tl + 1) * P)
                for c in range(2):
                    nc.tensor.matmul(
                        p2[:], actT_r[:, c, ns], w2r[:, c, :],
                        start=(c == 0), stop=False,
                    )
                nc.tensor.matmul(p2[:], ones_r[:], b2r[:], start=False, stop=True)
                out_sb = out_p.tile([P, H], F32, tag="out")
                nc.vector.tensor_tensor(
                    out=out_sb[:], in0=p2[:], in1=hh_tiles.pop(t)[:], op=ALU.add
                )
                nc.scalar.dma_start(out=h_d[t * P:(t + 1) * P, :], in_=out_sb[:])

        def coord_group(g0, g1):
            """coord for tiles [g0, g1): clip, mean over k, add clip(x)."""
            n = g1 - g0
            tr4 = trans_all[:, g0:g1, :].rearrange("p t (k c) -> p t k c", c=3)
            tc4 = crd_p.tile([P, n, K, 3], F32, tag="tc")
            nc.vector.tensor_scalar(
                out=tc4[:], in0=tr4, scalar1=-CLIP, scalar2=CLIP,
                op0=ALU.max, op1=ALU.min,
            )
            t8 = crd_p.tile([P, n, 8, 3], F32, tag="t8")
            nc.vector.tensor_tensor(
                out=t8[:], in0=tc4[:, :, 0:16:2, :], in1=tc4[:, :, 1:16:2, :], op=ALU.add
            )
            t4 = crd_p.tile([P, n, 4, 3], F32, tag="t4")
            nc.vector.tensor_tensor(
                out=t4[:], in0=t8[:, :, 0:8:2, :], in1=t8[:, :, 1:8:2, :], op=ALU.add
            )
            t2 = crd_p.tile([P, n, 2, 3], F32, tag="t2")
            nc.vector.tensor_tensor(
                out=t2[:], in0=t4[:, :, 0:4:2, :], in1=t4[:, :, 1:4:2, :], op=ALU.add
            )
            t1 = crd_p.tile([P, n, 3], F32, tag="t1")
            nc.vector.tensor_tensor(
                out=t1[:], in0=t2[:, :, 0, :], in1=t2[:, :, 1, :], op=ALU.add
            )
            nc.vector.tensor_scalar(
                out=t1[:], in0=t1[:], scalar1=1.0 / K, scalar2=None, op0=ALU.mult
            )
            xc = crd_p.tile([P, n, 3], F32, tag="xc")
            nc.vector.tensor_scalar(
                out=xc[:], in0=x_all[:, g0:g1, :], scalar1=-CLIP, scalar2=CLIP,
                op0=ALU.max, op1=ALU.min,
            )
            nc.vector.tensor_tensor(
                out=coord_all[:, g0:g1, :], in0=t1[:], in1=xc[:], op=ALU.add
            )

        # ---- main loop: macros of 2 tiles (last macro is 1 tile) ----
        for _rep in range(repeats):
            for m in range((T + 1) // 2):
                tiles = [t for t in (2 * m, 2 * m + 1) if t < T]
                aggT_r = agg_p.tile([P, 4, 2 * P], F32R, tag="aggT")
                for tl, t in enumerate(tiles):
                    load_and_reduce(t, aggT_r, tl)
                if variant != "dma":
                    mlp_and_store(tiles, aggT_r)
                    if m % 2 == 1:
                        coord_group(4 * (m // 2), min(4 * (m // 2) + 4, T))
            if variant != "dma" and T % 4 != 0:
                coord_group(T - T % 4, T)
        if variant == "dma":
            nc.vector.memset(coord_all[:], 0.0)

        nc.scalar.dma_start(
            out=co_d.rearrange("(t p) c -> p t c", p=P), in_=coord_all[:]
        )

    nc.compile()
    return nc


_NC_CACHE = []


def get_module():
    if not _NC_CACHE:
        _NC_CACHE.append(build_module())
    return _NC_CACHE[0]


def make_in_maps(x, trans, edge_feature, hh, W1, b1, W2, b2):
    ident = np.eye(P, dtype=np.float32)
    ones1 = np.ones((1, P), np.float32)
    in_maps = []
    for c in range(NCORES):
        sl = slice(c * NPC, (c + 1) * NPC)

        def pad(a):
            out = np.zeros((NPAD,) + a.shape[1:], np.float32)
            out[:NPC] = a[sl]
            return out

        in_maps.append({
            "x": pad(x), "trans": pad(trans), "edge": pad(edge_feature),
            "hh": pad(hh), "w1": np.ascontiguousarray(W1, np.float32),
            "b1": np.ascontiguousarray(b1, np.float32),
            "w2": np.ascontiguousarray(W2, np.float32),
            "b2": np.ascontiguousarray(b2, np.float32),
            "ident": ident, "ones1": ones1,
        })
    return in_maps


def kernel(x, trans, edge_feature, hh, W1, b1, W2, b2):
    nc = get_module()
    in_maps = make_in_maps(x, trans, edge_feature, hh, W1, b1, W2, b2)
    res = run_bass_kernel_spmd(nc, in_maps, core_ids=list(range(NCORES)))
    coord = np.empty((N, 3), np.float32)
    h = np.empty((N, H), np.float32)
    for c in range(NCORES):
        sl = slice(c * NPC, (c + 1) * NPC)
        coord[sl] = res.results[c]["coord"][:NPC]
        h[sl] = res.results[c]["hout"][:NPC]
    return coord, h
